# revision 31
# baseline (speedup 1.0000x reference)
"""MeshFC kernel for 8x TRN2 NeuronCores.

Computes: out = inputs @ w + biases, where
  w[i,o] = ||in_pos[i]-out_pos[o]|| - ||init_in_pos[i]-init_out_pos[o]||

Sharding: tensor-parallel on the output dim (8 x 1024 columns). Each core:
  - generates its weight column block on-chip via the PE using the
    augmented-inner-product identity dist^2 = ||a||^2 - 2 a.b + ||b||^2.
    The fp32 inner products are emulated with a bf16 hi/lo split
    (a = ah+al, b = bh+bl; a.b ~ ah.bh + ah.bl + al.bh, error ~2^-18)
    so the wg matmul is a single-pass bf16 stream (1 row/cycle) instead
    of a 2-pass fp32r stream that keeps the PE ~50% idle and the HAM
    clock-gate stuck at 1.2 GHz.
  - runs the main [4096,2048]x[2048,1024] matmul in fp16
  - biases are added on the Vector engine during the PSUM drain (a
    host-replicated [128,1024] broadcast tile), not via PE rank-1 matmuls
  - a burst of dummy warm-up matmuls at t=0 releases the HAM clock-gate
    (cold PE runs at 1.2 GHz; ~3.4us of sustained activity -> 2.4 GHz)
    while the input DMAs are still in flight.
Host side: pre-transposes/pre-tiles inputs so every DMA is contiguous,
and concatenates the 8 per-core [4096,1024] outputs.
"""

import os
from contextlib import ExitStack

import numpy as np

NUM_IN, NUM_OUT, SD, BATCH = 2048, 8192, 5, 4096
N_CORES = 8
O_SHARD = NUM_OUT // N_CORES  # 1024
B_TILES = BATCH // 128  # 32
K_TILES = NUM_IN // 128  # 16
O_HALves = O_SHARD // 512  # 2
# 36 rows: 6 bf16 cross-product blocks (ah.bh, ah.bl, al.bh, ah.bl2, al.bl,
# al2.bh) + 3-term hi/lo/lo2 splits of ||a||^2 and ||b||^2 -> dist^2 with
# ~2^-27 split error (fp32-class), in a single 1-row/cycle bf16 PE pass.
KAUG = 6 * SD + 6  # 36
N_WARM = 10
HB = 16  # batch-tile half: x tiles resident per half, reused across oh groups

_CACHE = {}


def _build_bass(variant=""):
    import concourse.bass as bass  # noqa: F401
    import concourse.mybir as mybir
    from concourse import bacc
    from concourse.tile import TileContext

    fp32 = mybir.dt.float32
    bf16 = mybir.dt.bfloat16
    fp16 = mybir.dt.float16

    mmdt = fp16

    # Bacc (not plain Bass): its compile() runs generate_event_semaphores +
    # move_matmul_waits_to_ldweights, which split multi-waits that exceed the
    # per-instruction HW sync-wait budget.
    nc = bacc.Bacc("TRN2", name="meshfc")

    xT = nc.dram_tensor("xT", [B_TILES, 128, NUM_IN], mmdt, kind="ExternalInput")
    # packed [A_C | A_I | B_C | B_I] along the free axis -> single DMA
    AB_W = 2 * NUM_IN + 2 * O_SHARD
    ab = nc.dram_tensor("ab", [KAUG, AB_W], bf16, kind="ExternalInput")
    # bias replicated to 128 partitions host-side; added on DVE during drain
    bias = nc.dram_tensor("bias", [128, O_SHARD], fp32, kind="ExternalInput")
    out = nc.dram_tensor("out", [BATCH, O_SHARD], fp32, kind="ExternalOutput")

    with ExitStack() as ctx:
        tc = ctx.enter_context(TileContext(nc))
        const = ctx.enter_context(tc.tile_pool(name="const", bufs=1))
        wps = ctx.enter_context(tc.tile_pool(name="wps", bufs=2, space="PSUM"))
        tmp = ctx.enter_context(tc.tile_pool(name="tmp", bufs=2))
        mps = ctx.enter_context(tc.tile_pool(name="mps", bufs=2, space="PSUM"))
        xpool = ctx.enter_context(tc.tile_pool(name="xp", bufs=HB))
        opool = ctx.enter_context(tc.tile_pool(name="op", bufs=3))

        # --- PE warm-up: release the HAM clock-gate during input DMA ---
        # (psum tiles come from the main-matmul pool; warmups finish long
        # before the first main MM reuses those banks)
        warm_sb = const.tile([128, 512], bf16, name="warm_sb")
        nc.vector.memset(warm_sb, 0.0)
        # prime ScalarE's sqrt activation table and GpSimd's tensor-op
        # library on tiny tiles now, so the first-use setup cost lands here
        # instead of inside the latency-critical weight-gen chain
        prime_sb = const.tile([1, 8], fp32, name="prime_sb")
        nc.vector.memset(prime_sb, 1.0)
        nc.scalar.sqrt(prime_sb[0:1, 0:4], prime_sb[0:1, 0:4])
        nc.gpsimd.tensor_sub(prime_sb[0:1, 4:8], prime_sb[0:1, 0:4], prime_sb[0:1, 0:4])

        def dummy_mm():
            wp = mps.tile([128, 512], fp32, tag="ps", bufs=4)
            nc.tensor.matmul(wp, warm_sb[:, 0:128], warm_sb, start=True, stop=True)

        for _ in range(N_WARM):
            dummy_mm()

        # --- constants ---
        ab_sb = const.tile([KAUG, AB_W], bf16, name="ab_sb")
        nc.sync.dma_start(out=ab_sb, in_=ab[:, :])
        aC_sb = ab_sb[:, 0:NUM_IN]
        aI_sb = ab_sb[:, NUM_IN : 2 * NUM_IN]
        bC_sb = ab_sb[:, 2 * NUM_IN : 2 * NUM_IN + O_SHARD]
        bI_sb = ab_sb[:, 2 * NUM_IN + O_SHARD : AB_W]

        # bias DMA is issued after the first xT tiles (see below): it is
        # first read at ~37us, while the chain-tracking matmuls need their
        # x tiles at ~14-16us on the same DMA queue
        bias_sb = const.tile([128, O_SHARD], fp32, name="bias_sb")

        # resident weight block: [128, K_TILES, O_SHARD] = 4 MB fp16
        w_sb = const.tile([128, K_TILES, O_SHARD], mmdt, name="w_sb")

        def wg_iter(oh, kt, n_dummy):
            """One weight-gen tile: dist^2 matmuls + clamp/sqrt/sub chain.

            The elementwise chain is throughput-bound at ~1.2us/tile on DVE
            (2 maxes), ScalarE (2 sqrts) and GpSimd (sub) simultaneously,
            while the two PE matmuls take only ~0.43us - n_dummy filler MMs
            keep the PE near 100% duty so the HAM clock-gate stays released.
            """
            osl = slice(oh * 512, (oh + 1) * 512)
            ksl = slice(kt * 128, (kt + 1) * 128)
            psC = wps.tile([128, 512], fp32, tag="psC", bufs=2)
            psI = wps.tile([128, 512], fp32, tag="psI", bufs=2)
            nc.tensor.matmul(psC, aC_sb[:, ksl], bC_sb[:, osl], start=True, stop=True)
            nc.tensor.matmul(psI, aI_sb[:, ksl], bI_sb[:, osl], start=True, stop=True)
            for _ in range(n_dummy):
                dummy_mm()
            # clamp dist^2 to >=0 on DVE (fp32 rounding can push the closest
            # pair slightly negative -> sqrt NaN), then sqrt in SBUF.
            # In-place PSUM activation crashes the exec unit; a 1024-wide
            # ACTIVATE is NRT_EXEC_UNIT_UNRECOVERABLE - keep 512-wide ops.
            sC = tmp.tile([128, 512], fp32, tag="sC", bufs=2)
            sI = tmp.tile([128, 512], fp32, tag="sI", bufs=2)
            nc.vector.tensor_scalar_max(sC, psC, 0.0)
            nc.vector.tensor_scalar_max(sI, psI, 0.0)
            nc.scalar.sqrt(sC, sC)
            nc.scalar.sqrt(sI, sI)
            # sub on GpSimd: on the in-order DVE it would wait for the
            # ScalarE sqrts and head-of-line-block the next iteration's
            # max, throttling wg PSUM recycling to sqrt pace
            nc.gpsimd.tensor_sub(w_sb[:, kt, osl], sC, sI)

        def main_group(oh, bt, xt):
            """16-MM PSUM accumulation for one [128,512] output tile."""
            osl = slice(oh * 512, (oh + 1) * 512)
            ot = opool.tile([128, 512], fp32, name="ot")
            # pre-touch: absorbs the out-DMA slot-release wait on ScalarE
            # so the drains stay within the HW sync-wait slot budget
            # (GpSimd tried and reverted: its per-instruction overhead
            # delays the dependent DVE drain and stalls PSUM recycling)
            nc.scalar.mul(ot[0:1, 0:1], ot[0:1, 0:1], 0.0)
            ps = mps.tile([128, 512], fp32, tag="ps", bufs=4)
            for kt in range(K_TILES):
                nc.tensor.matmul(
                    ps,
                    xt[:, kt * 128 : (kt + 1) * 128],
                    w_sb[:, kt, osl],
                    start=(kt == 0),
                    stop=(kt == K_TILES - 1),
                )
            # drain with bias add on DVE (was: PE matmul + scalar copy)
            nc.vector.tensor_add(ot, ps, bias_sb[:, osl])
            nc.sync.dma_start(out=out[bt * 128 : (bt + 1) * 128, osl], in_=ot)

        def x_tiles(half):
            xts = []
            for i in range(HB):
                xt = xpool.tile([128, NUM_IN], mmdt, name="xt")
                nc.sync.dma_start(out=xt, in_=xT[half * HB + i])
                xts.append(xt)
            return xts

        # Issue order is the PE execution order (in-order engine queues):
        #  1. wg oh=0 with TWO batch-tiles' oh=0 accumulations tracking the
        #     chain per-kt: each kt matmul is issued right after that kt's
        #     weight tile, so real main work (not dummy fillers) keeps the
        #     PE busy while the sqrt chain drains. Two trackers keep the
        #     phase chain-paced (852+432 < 1400ns/iter) and within the xT
        #     DMA arrival ramp; their PSUM tiles stay open for the phase.
        #  2. wg oh=1 interleaved with the remaining half0/oh=0 groups -
        #     the oh=0 groups only need oh=0 weights (no HOL blocking)
        #  3. remaining main groups (all weights ready by then)
        N_TRACK = 3
        xts0 = x_tiles(0)
        nc.sync.dma_start(out=bias_sb, in_=bias[:, :])
        otA, psA = [], []
        for b in range(N_TRACK):
            ot = opool.tile([128, 512], fp32, name="ot")
            # pre-touch issued before the chain: runs while ScalarE is idle
            nc.scalar.mul(ot[0:1, 0:1], ot[0:1, 0:1], 0.0)
            otA.append(ot)
            ps_b = mps.tile([128, 512], fp32, tag="ps", bufs=4, name="psA")
            psA.append(ps_b)
        for j in range(K_TILES):
            # extra fillers on the first iterations bridge the chain's
            # pipeline-fill latency (~4us) so HAM never sees an idle window
            wg_iter(0, j, n_dummy=5 if j < 2 else 0)
            for b in range(N_TRACK):
                nc.tensor.matmul(
                    psA[b],
                    xts0[b][:, j * 128 : (j + 1) * 128],
                    w_sb[:, j, 0:512],
                    start=(j == 0),
                    stop=(j == K_TILES - 1),
                )
        for b in range(N_TRACK):
            nc.vector.tensor_add(otA[b], psA[b], bias_sb[:, 0:512])
            nc.sync.dma_start(out=out[b * 128 : (b + 1) * 128, 0:512], in_=otA[b])
        n_pair = HB - N_TRACK  # 13 oh=0 groups left to pair with wg oh=1
        for j in range(n_pair):
            wg_iter(1, j, n_dummy=0)
            main_group(0, N_TRACK + j, xts0[N_TRACK + j])
        for j in range(n_pair, K_TILES):
            wg_iter(1, j, n_dummy=2)
        for i in range(HB):
            main_group(1, i, xts0[i])
        xts1 = x_tiles(1)
        for oh in range(O_HALves):
            for i in range(HB):
                main_group(oh, HB + i, xts1[i])

    nc.finalize()
    return nc


def _hi_lo3(v):
    """Split fp32 array into 3 bf16 terms with v ~ h+l+l2 to ~2^-27."""
    import ml_dtypes

    bf = ml_dtypes.bfloat16
    h = v.astype(bf)
    r = v - h.astype(np.float32)
    l = r.astype(bf)
    l2 = (r - l.astype(np.float32)).astype(bf)
    return h, l, l2


def _aug_pair(p, q):
    """Augmented bf16 row blocks for a (in, [N,5]) and b (out, [M,5]) such
    that A.T @ B ~= ||a||^2 - 2 a.b + ||b||^2 in one bf16 matmul pass with
    ~2^-27 split error (the residual error is fp32 PSUM accumulation).

    a ~ ah+al+al2 (bf16 each); kept cross terms: ah.bh, ah.bl, al.bh,
    ah.bl2, al.bl, al2.bh; norms split 3-way against ones rows.
    """
    import ml_dtypes

    bf = ml_dtypes.bfloat16
    n, m = p.shape[0], q.shape[0]
    na = (p.astype(np.float64) ** 2).sum(1).astype(np.float32)
    nb = (q.astype(np.float64) ** 2).sum(1).astype(np.float32)
    ah, al, al2 = _hi_lo3(p)
    bh, bl, bl2 = _hi_lo3(q)
    nah, nal, nal2 = _hi_lo3(na)
    nbh, nbl, nbl2 = _hi_lo3(nb)
    ones_n = np.ones((1, n), bf)
    ones_m = np.ones((1, m), bf)
    A = np.concatenate(
        [ah.T, ah.T, al.T, ah.T, al.T, al2.T,
         nah[None, :], nal[None, :], nal2[None, :], ones_n, ones_n, ones_n], 0
    ).astype(bf)
    B = np.concatenate(
        [-2 * bh.T, -2 * bl.T, -2 * bh.T, -2 * bl2.T, -2 * bl.T, -2 * bh.T,
         ones_m, ones_m, ones_m, nbh[None, :], nbl[None, :], nbl2[None, :]], 0
    ).astype(bf)
    assert A.shape == (KAUG, n) and B.shape == (KAUG, m)
    return A, B


def _prep_inputs(inputs, init_in_pos, init_out_pos, in_pos, out_pos, biases,
                 mm_np_dt=np.float16):
    x = np.ascontiguousarray(np.asarray(inputs, dtype=np.float32))
    a = np.asarray(in_pos, dtype=np.float32).reshape(NUM_IN, SD)
    a0 = np.asarray(init_in_pos, dtype=np.float32).reshape(NUM_IN, SD)
    b = np.asarray(out_pos, dtype=np.float32).reshape(NUM_OUT, SD)
    b0 = np.asarray(init_out_pos, dtype=np.float32).reshape(NUM_OUT, SD)
    bias = np.asarray(biases, dtype=np.float32).reshape(NUM_OUT)

    # [bt, p, kt*128+b'] = x[bt*128+b', kt*128+p]
    xT = np.ascontiguousarray(
        x.reshape(B_TILES, 128, K_TILES, 128).transpose(0, 3, 2, 1).astype(mm_np_dt)
    ).reshape(B_TILES, 128, NUM_IN)

    A_C, B_C_full = _aug_pair(a, b)
    A_I, B_I_full = _aug_pair(a0, b0)

    in_maps = []
    for c in range(N_CORES):
        sl = slice(c * O_SHARD, (c + 1) * O_SHARD)
        ab = np.ascontiguousarray(
            np.concatenate([A_C, A_I, B_C_full[:, sl], B_I_full[:, sl]], axis=1)
        )
        bias_bc = np.ascontiguousarray(
            np.broadcast_to(bias[sl][None, :], (128, O_SHARD)).astype(np.float32)
        )
        in_maps.append({"xT": xT, "ab": ab, "bias": bias_bc})
    return in_maps


def _run(in_maps, trace=False):
    from concourse.bass_utils import run_bass_kernel_spmd

    if "nc" not in _CACHE:
        _CACHE["nc"] = _build_bass()
    nc = _CACHE["nc"]
    res = run_bass_kernel_spmd(
        nc, in_maps, core_ids=list(range(N_CORES)), trace=trace
    )
    outs = [r["out"] for r in res.results]
    return np.concatenate(outs, axis=1), res


def kernel(**inputs) -> np.ndarray:
    in_maps = _prep_inputs(**inputs)
    out, _ = _run(in_maps, trace=bool(os.environ.get("MESHFC_TRACE")))
    return out


# revision 32
# speedup vs baseline: 1.0225x; 1.0225x over previous
"""MeshFC kernel for 8x TRN2 NeuronCores.

Computes: out = inputs @ w + biases, where
  w[i,o] = ||in_pos[i]-out_pos[o]|| - ||init_in_pos[i]-init_out_pos[o]||

Sharding: tensor-parallel on the output dim (8 x 1024 columns). Each core:
  - generates its weight column block on-chip via the PE using the
    augmented-inner-product identity dist^2 = ||a||^2 - 2 a.b + ||b||^2.
    The fp32 inner products are emulated with a bf16 hi/lo split
    (a = ah+al, b = bh+bl; a.b ~ ah.bh + ah.bl + al.bh, error ~2^-18)
    so the wg matmul is a single-pass bf16 stream (1 row/cycle) instead
    of a 2-pass fp32r stream that keeps the PE ~50% idle and the HAM
    clock-gate stuck at 1.2 GHz.
  - runs the main [4096,2048]x[2048,1024] matmul in fp16
  - biases are added on the Vector engine during the PSUM drain (a
    host-replicated [128,1024] broadcast tile), not via PE rank-1 matmuls
  - a burst of dummy warm-up matmuls at t=0 releases the HAM clock-gate
    (cold PE runs at 1.2 GHz; ~3.4us of sustained activity -> 2.4 GHz)
    while the input DMAs are still in flight.
Host side: pre-transposes/pre-tiles inputs so every DMA is contiguous,
and concatenates the 8 per-core [4096,1024] outputs.
"""

import os
from contextlib import ExitStack

import numpy as np

NUM_IN, NUM_OUT, SD, BATCH = 2048, 8192, 5, 4096
N_CORES = 8
O_SHARD = NUM_OUT // N_CORES  # 1024
B_TILES = BATCH // 128  # 32
K_TILES = NUM_IN // 128  # 16
O_HALves = O_SHARD // 512  # 2
# 36 rows: 6 bf16 cross-product blocks (ah.bh, ah.bl, al.bh, ah.bl2, al.bl,
# al2.bh) + 3-term hi/lo/lo2 splits of ||a||^2 and ||b||^2 -> dist^2 with
# ~2^-27 split error (fp32-class), in a single 1-row/cycle bf16 PE pass.
KAUG = 6 * SD + 6  # 36
N_WARM = 10
HB = 16  # batch-tile half: x tiles resident per half, reused across oh groups

_CACHE = {}


def _build_bass(variant=""):
    import concourse.bass as bass  # noqa: F401
    import concourse.mybir as mybir
    from concourse import bacc
    from concourse.tile import TileContext

    fp32 = mybir.dt.float32
    bf16 = mybir.dt.bfloat16
    fp16 = mybir.dt.float16

    mmdt = fp16

    # Bacc (not plain Bass): its compile() runs generate_event_semaphores +
    # move_matmul_waits_to_ldweights, which split multi-waits that exceed the
    # per-instruction HW sync-wait budget.
    nc = bacc.Bacc("TRN2", name="meshfc")

    xT = nc.dram_tensor("xT", [B_TILES, 128, NUM_IN], mmdt, kind="ExternalInput")
    # packed [A_C | A_I | B_C | B_I] along the free axis -> single DMA
    AB_W = 2 * NUM_IN + 2 * O_SHARD
    ab = nc.dram_tensor("ab", [KAUG, AB_W], bf16, kind="ExternalInput")
    # bias replicated to 128 partitions host-side; added on DVE during drain
    bias = nc.dram_tensor("bias", [128, O_SHARD], fp32, kind="ExternalInput")
    out = nc.dram_tensor("out", [BATCH, O_SHARD], fp32, kind="ExternalOutput")

    with ExitStack() as ctx:
        tc = ctx.enter_context(TileContext(nc))
        const = ctx.enter_context(tc.tile_pool(name="const", bufs=1))
        wps = ctx.enter_context(tc.tile_pool(name="wps", bufs=2, space="PSUM"))
        tmp = ctx.enter_context(tc.tile_pool(name="tmp", bufs=2))
        mps = ctx.enter_context(tc.tile_pool(name="mps", bufs=2, space="PSUM"))
        xpool = ctx.enter_context(tc.tile_pool(name="xp", bufs=HB))
        opool = ctx.enter_context(tc.tile_pool(name="op", bufs=3))

        # --- PE warm-up: release the HAM clock-gate during input DMA ---
        # (psum tiles come from the main-matmul pool; warmups finish long
        # before the first main MM reuses those banks)
        warm_sb = const.tile([128, 512], bf16, name="warm_sb")
        nc.vector.memset(warm_sb, 0.0)
        # prime ScalarE's sqrt activation table and GpSimd's tensor-op
        # library on tiny tiles now, so the first-use setup cost lands here
        # instead of inside the latency-critical weight-gen chain
        prime_sb = const.tile([1, 8], fp32, name="prime_sb")
        nc.vector.memset(prime_sb, 1.0)
        nc.scalar.sqrt(prime_sb[0:1, 0:4], prime_sb[0:1, 0:4])
        nc.gpsimd.tensor_sub(prime_sb[0:1, 4:8], prime_sb[0:1, 0:4], prime_sb[0:1, 0:4])

        def dummy_mm():
            wp = mps.tile([128, 512], fp32, tag="ps", bufs=4)
            nc.tensor.matmul(wp, warm_sb[:, 0:128], warm_sb, start=True, stop=True)

        for _ in range(N_WARM):
            dummy_mm()

        # --- constants ---
        ab_sb = const.tile([KAUG, AB_W], bf16, name="ab_sb")
        nc.sync.dma_start(out=ab_sb, in_=ab[:, :])
        aC_sb = ab_sb[:, 0:NUM_IN]
        aI_sb = ab_sb[:, NUM_IN : 2 * NUM_IN]
        bC_sb = ab_sb[:, 2 * NUM_IN : 2 * NUM_IN + O_SHARD]
        bI_sb = ab_sb[:, 2 * NUM_IN + O_SHARD : AB_W]

        bias_sb = const.tile([128, O_SHARD], fp32, name="bias_sb")
        nc.sync.dma_start(out=bias_sb, in_=bias[:, :])

        # resident weight block: [128, K_TILES, O_SHARD] = 4 MB fp16
        w_sb = const.tile([128, K_TILES, O_SHARD], mmdt, name="w_sb")

        def wg_iter(oh, kt, n_dummy):
            """One weight-gen tile: dist^2 matmuls + clamp/sqrt/sub chain.

            The elementwise chain is throughput-bound at ~1.2us/tile on DVE
            (2 maxes), ScalarE (2 sqrts) and GpSimd (sub) simultaneously,
            while the two PE matmuls take only ~0.43us - n_dummy filler MMs
            keep the PE near 100% duty so the HAM clock-gate stays released.
            """
            osl = slice(oh * 512, (oh + 1) * 512)
            ksl = slice(kt * 128, (kt + 1) * 128)
            psC = wps.tile([128, 512], fp32, tag="psC", bufs=2)
            psI = wps.tile([128, 512], fp32, tag="psI", bufs=2)
            nc.tensor.matmul(psC, aC_sb[:, ksl], bC_sb[:, osl], start=True, stop=True)
            nc.tensor.matmul(psI, aI_sb[:, ksl], bI_sb[:, osl], start=True, stop=True)
            for _ in range(n_dummy):
                dummy_mm()
            # clamp dist^2 to >=0 on DVE (fp32 rounding can push the closest
            # pair slightly negative -> sqrt NaN), then sqrt in SBUF.
            # In-place PSUM activation crashes the exec unit; a 1024-wide
            # ACTIVATE is NRT_EXEC_UNIT_UNRECOVERABLE - keep 512-wide ops.
            sC = tmp.tile([128, 512], fp32, tag="sC", bufs=2)
            sI = tmp.tile([128, 512], fp32, tag="sI", bufs=2)
            nc.vector.tensor_scalar_max(sC, psC, 0.0)
            nc.vector.tensor_scalar_max(sI, psI, 0.0)
            nc.scalar.sqrt(sC, sC)
            nc.scalar.sqrt(sI, sI)
            # sub on GpSimd: on the in-order DVE it would wait for the
            # ScalarE sqrts and head-of-line-block the next iteration's
            # max, throttling wg PSUM recycling to sqrt pace
            nc.gpsimd.tensor_sub(w_sb[:, kt, osl], sC, sI)

        def main_group(oh, bt, xt):
            """16-MM PSUM accumulation for one [128,512] output tile."""
            osl = slice(oh * 512, (oh + 1) * 512)
            ot = opool.tile([128, 512], fp32, name="ot")
            # pre-touch: absorbs the out-DMA slot-release wait on ScalarE
            # so the drains stay within the HW sync-wait slot budget
            # (GpSimd tried and reverted: its per-instruction overhead
            # delays the dependent DVE drain and stalls PSUM recycling)
            nc.scalar.mul(ot[0:1, 0:1], ot[0:1, 0:1], 0.0)
            ps = mps.tile([128, 512], fp32, tag="ps", bufs=4)
            for kt in range(K_TILES):
                nc.tensor.matmul(
                    ps,
                    xt[:, kt * 128 : (kt + 1) * 128],
                    w_sb[:, kt, osl],
                    start=(kt == 0),
                    stop=(kt == K_TILES - 1),
                )
            # drain with bias add on DVE (was: PE matmul + scalar copy)
            nc.vector.tensor_add(ot, ps, bias_sb[:, osl])
            nc.sync.dma_start(out=out[bt * 128 : (bt + 1) * 128, osl], in_=ot)

        def x_tiles(half):
            xts = []
            for i in range(HB):
                xt = xpool.tile([128, NUM_IN], mmdt, name="xt")
                nc.sync.dma_start(out=xt, in_=xT[half * HB + i])
                xts.append(xt)
            return xts

        # Issue order is the PE execution order (in-order engine queues):
        #  1. wg oh=0, PE duty padded with filler MMs
        #  2. wg oh=1 interleaved with half0/oh=0 main groups - the main MMs
        #     fill the PE while the oh=1 elementwise chain drains, and the
        #     oh=0 groups only need oh=0 weights (no head-of-line blocking)
        #  3. remaining main groups (all weights ready by then)
        xts0 = x_tiles(0)
        for kt in range(K_TILES):
            wg_iter(0, kt, n_dummy=1)
        for j in range(K_TILES):
            wg_iter(1, j, n_dummy=2 if j < 3 else 0)
            main_group(0, j, xts0[j])
        for i in range(HB):
            main_group(1, i, xts0[i])
        xts1 = x_tiles(1)
        for oh in range(O_HALves):
            for i in range(HB):
                main_group(oh, HB + i, xts1[i])

    nc.finalize()
    return nc


def _hi_lo3(v):
    """Split fp32 array into 3 bf16 terms with v ~ h+l+l2 to ~2^-27."""
    import ml_dtypes

    bf = ml_dtypes.bfloat16
    h = v.astype(bf)
    r = v - h.astype(np.float32)
    l = r.astype(bf)
    l2 = (r - l.astype(np.float32)).astype(bf)
    return h, l, l2


def _aug_pair(p, q):
    """Augmented bf16 row blocks for a (in, [N,5]) and b (out, [M,5]) such
    that A.T @ B ~= ||a||^2 - 2 a.b + ||b||^2 in one bf16 matmul pass with
    ~2^-27 split error (the residual error is fp32 PSUM accumulation).

    a ~ ah+al+al2 (bf16 each); kept cross terms: ah.bh, ah.bl, al.bh,
    ah.bl2, al.bl, al2.bh; norms split 3-way against ones rows.
    """
    import ml_dtypes

    bf = ml_dtypes.bfloat16
    n, m = p.shape[0], q.shape[0]
    na = (p.astype(np.float64) ** 2).sum(1).astype(np.float32)
    nb = (q.astype(np.float64) ** 2).sum(1).astype(np.float32)
    ah, al, al2 = _hi_lo3(p)
    bh, bl, bl2 = _hi_lo3(q)
    nah, nal, nal2 = _hi_lo3(na)
    nbh, nbl, nbl2 = _hi_lo3(nb)
    ones_n = np.ones((1, n), bf)
    ones_m = np.ones((1, m), bf)
    A = np.concatenate(
        [ah.T, ah.T, al.T, ah.T, al.T, al2.T,
         nah[None, :], nal[None, :], nal2[None, :], ones_n, ones_n, ones_n], 0
    ).astype(bf)
    B = np.concatenate(
        [-2 * bh.T, -2 * bl.T, -2 * bh.T, -2 * bl2.T, -2 * bl.T, -2 * bh.T,
         ones_m, ones_m, ones_m, nbh[None, :], nbl[None, :], nbl2[None, :]], 0
    ).astype(bf)
    assert A.shape == (KAUG, n) and B.shape == (KAUG, m)
    return A, B


def _prep_inputs(inputs, init_in_pos, init_out_pos, in_pos, out_pos, biases,
                 mm_np_dt=np.float16):
    x = np.ascontiguousarray(np.asarray(inputs, dtype=np.float32))
    a = np.asarray(in_pos, dtype=np.float32).reshape(NUM_IN, SD)
    a0 = np.asarray(init_in_pos, dtype=np.float32).reshape(NUM_IN, SD)
    b = np.asarray(out_pos, dtype=np.float32).reshape(NUM_OUT, SD)
    b0 = np.asarray(init_out_pos, dtype=np.float32).reshape(NUM_OUT, SD)
    bias = np.asarray(biases, dtype=np.float32).reshape(NUM_OUT)

    # [bt, p, kt*128+b'] = x[bt*128+b', kt*128+p]
    xT = np.ascontiguousarray(
        x.reshape(B_TILES, 128, K_TILES, 128).transpose(0, 3, 2, 1).astype(mm_np_dt)
    ).reshape(B_TILES, 128, NUM_IN)

    A_C, B_C_full = _aug_pair(a, b)
    A_I, B_I_full = _aug_pair(a0, b0)

    in_maps = []
    for c in range(N_CORES):
        sl = slice(c * O_SHARD, (c + 1) * O_SHARD)
        ab = np.ascontiguousarray(
            np.concatenate([A_C, A_I, B_C_full[:, sl], B_I_full[:, sl]], axis=1)
        )
        bias_bc = np.ascontiguousarray(
            np.broadcast_to(bias[sl][None, :], (128, O_SHARD)).astype(np.float32)
        )
        in_maps.append({"xT": xT, "ab": ab, "bias": bias_bc})
    return in_maps


def _run(in_maps, trace=False):
    from concourse.bass_utils import run_bass_kernel_spmd

    if "nc" not in _CACHE:
        _CACHE["nc"] = _build_bass()
    nc = _CACHE["nc"]
    res = run_bass_kernel_spmd(
        nc, in_maps, core_ids=list(range(N_CORES)), trace=trace
    )
    outs = [r["out"] for r in res.results]
    return np.concatenate(outs, axis=1), res


def kernel(**inputs) -> np.ndarray:
    in_maps = _prep_inputs(**inputs)
    out, _ = _run(in_maps, trace=bool(os.environ.get("MESHFC_TRACE")))
    return out


# revision 34
# speedup vs baseline: 1.0258x; 1.0031x over previous
"""MeshFC kernel for 8x TRN2 NeuronCores.

Computes: out = inputs @ w + biases, where
  w[i,o] = ||in_pos[i]-out_pos[o]|| - ||init_in_pos[i]-init_out_pos[o]||

Sharding: tensor-parallel on the output dim (8 x 1024 columns). Each core:
  - generates its weight column block on-chip via the PE using the
    augmented-inner-product identity dist^2 = ||a||^2 - 2 a.b + ||b||^2.
    The fp32 inner products are emulated with a bf16 hi/lo split
    (a = ah+al, b = bh+bl; a.b ~ ah.bh + ah.bl + al.bh, error ~2^-18)
    so the wg matmul is a single-pass bf16 stream (1 row/cycle) instead
    of a 2-pass fp32r stream that keeps the PE ~50% idle and the HAM
    clock-gate stuck at 1.2 GHz.
  - runs the main [4096,2048]x[2048,1024] matmul in fp16
  - biases are added on the Vector engine during the PSUM drain (a
    host-replicated [128,1024] broadcast tile), not via PE rank-1 matmuls
  - a burst of dummy warm-up matmuls at t=0 releases the HAM clock-gate
    (cold PE runs at 1.2 GHz; ~3.4us of sustained activity -> 2.4 GHz)
    while the input DMAs are still in flight.
Host side: pre-transposes/pre-tiles inputs so every DMA is contiguous,
and concatenates the 8 per-core [4096,1024] outputs.
"""

import os
from contextlib import ExitStack

import numpy as np

NUM_IN, NUM_OUT, SD, BATCH = 2048, 8192, 5, 4096
N_CORES = 8
O_SHARD = NUM_OUT // N_CORES  # 1024
B_TILES = BATCH // 128  # 32
K_TILES = NUM_IN // 128  # 16
O_HALves = O_SHARD // 512  # 2
# 36 rows: 6 bf16 cross-product blocks (ah.bh, ah.bl, al.bh, ah.bl2, al.bl,
# al2.bh) + 3-term hi/lo/lo2 splits of ||a||^2 and ||b||^2 -> dist^2 with
# ~2^-27 split error (fp32-class), in a single 1-row/cycle bf16 PE pass.
KAUG = 6 * SD + 6  # 36
N_WARM = 9
HB = 16  # batch-tile half: x tiles resident per half, reused across oh groups

_CACHE = {}


def _build_bass(variant=""):
    import concourse.bass as bass  # noqa: F401
    import concourse.mybir as mybir
    from concourse import bacc
    from concourse.tile import TileContext

    fp32 = mybir.dt.float32
    bf16 = mybir.dt.bfloat16
    fp16 = mybir.dt.float16

    mmdt = fp16

    # Bacc (not plain Bass): its compile() runs generate_event_semaphores +
    # move_matmul_waits_to_ldweights, which split multi-waits that exceed the
    # per-instruction HW sync-wait budget.
    nc = bacc.Bacc("TRN2", name="meshfc")

    xT = nc.dram_tensor("xT", [B_TILES, 128, NUM_IN], mmdt, kind="ExternalInput")
    # packed [A_C | A_I | B_C | B_I] along the free axis -> single DMA
    AB_W = 2 * NUM_IN + 2 * O_SHARD
    ab = nc.dram_tensor("ab", [KAUG, AB_W], bf16, kind="ExternalInput")
    # bias replicated to 128 partitions host-side; added on DVE during drain
    bias = nc.dram_tensor("bias", [128, O_SHARD], fp32, kind="ExternalInput")
    out = nc.dram_tensor("out", [BATCH, O_SHARD], fp32, kind="ExternalOutput")

    with ExitStack() as ctx:
        tc = ctx.enter_context(TileContext(nc))
        const = ctx.enter_context(tc.tile_pool(name="const", bufs=1))
        wps = ctx.enter_context(tc.tile_pool(name="wps", bufs=2, space="PSUM"))
        tmp = ctx.enter_context(tc.tile_pool(name="tmp", bufs=2))
        mps = ctx.enter_context(tc.tile_pool(name="mps", bufs=2, space="PSUM"))
        xpool = ctx.enter_context(tc.tile_pool(name="xp", bufs=HB))
        opool = ctx.enter_context(tc.tile_pool(name="op", bufs=3))

        # --- PE warm-up: release the HAM clock-gate during input DMA ---
        # (psum tiles come from the main-matmul pool; warmups finish long
        # before the first main MM reuses those banks)
        warm_sb = const.tile([128, 512], bf16, name="warm_sb")
        nc.vector.memset(warm_sb, 0.0)
        # prime ScalarE's sqrt activation table and GpSimd's tensor-op
        # library on tiny tiles now, so the first-use setup cost lands here
        # instead of inside the latency-critical weight-gen chain
        prime_sb = const.tile([1, 8], fp32, name="prime_sb")
        nc.vector.memset(prime_sb, 1.0)
        nc.scalar.sqrt(prime_sb[0:1, 0:4], prime_sb[0:1, 0:4])
        nc.gpsimd.tensor_sub(prime_sb[0:1, 4:8], prime_sb[0:1, 0:4], prime_sb[0:1, 0:4])

        def dummy_mm():
            wp = mps.tile([128, 512], fp32, tag="ps", bufs=4)
            nc.tensor.matmul(wp, warm_sb[:, 0:128], warm_sb, start=True, stop=True)

        for _ in range(N_WARM):
            dummy_mm()

        # --- constants ---
        ab_sb = const.tile([KAUG, AB_W], bf16, name="ab_sb")
        nc.sync.dma_start(out=ab_sb, in_=ab[:, :])
        aC_sb = ab_sb[:, 0:NUM_IN]
        aI_sb = ab_sb[:, NUM_IN : 2 * NUM_IN]
        bC_sb = ab_sb[:, 2 * NUM_IN : 2 * NUM_IN + O_SHARD]
        bI_sb = ab_sb[:, 2 * NUM_IN + O_SHARD : AB_W]

        bias_sb = const.tile([128, O_SHARD], fp32, name="bias_sb")
        nc.sync.dma_start(out=bias_sb, in_=bias[:, :])

        # resident weight block: [128, K_TILES, O_SHARD] = 4 MB fp16
        w_sb = const.tile([128, K_TILES, O_SHARD], mmdt, name="w_sb")

        def wg_iter(oh, kt, n_dummy):
            """One weight-gen tile: dist^2 matmuls + clamp/sqrt/sub chain.

            The elementwise chain is throughput-bound at ~1.2us/tile on DVE
            (2 maxes), ScalarE (2 sqrts) and GpSimd (sub) simultaneously,
            while the two PE matmuls take only ~0.43us - n_dummy filler MMs
            keep the PE near 100% duty so the HAM clock-gate stays released.
            """
            osl = slice(oh * 512, (oh + 1) * 512)
            ksl = slice(kt * 128, (kt + 1) * 128)
            psC = wps.tile([128, 512], fp32, tag="psC", bufs=2)
            psI = wps.tile([128, 512], fp32, tag="psI", bufs=2)
            nc.tensor.matmul(psC, aC_sb[:, ksl], bC_sb[:, osl], start=True, stop=True)
            nc.tensor.matmul(psI, aI_sb[:, ksl], bI_sb[:, osl], start=True, stop=True)
            for _ in range(n_dummy):
                dummy_mm()
            # clamp dist^2 to >=0 on DVE (fp32 rounding can push the closest
            # pair slightly negative -> sqrt NaN), then sqrt in SBUF.
            # In-place PSUM activation crashes the exec unit; a 1024-wide
            # ACTIVATE is NRT_EXEC_UNIT_UNRECOVERABLE - keep 512-wide ops.
            sC = tmp.tile([128, 512], fp32, tag="sC", bufs=2)
            sI = tmp.tile([128, 512], fp32, tag="sI", bufs=2)
            nc.vector.tensor_scalar_max(sC, psC, 0.0)
            nc.vector.tensor_scalar_max(sI, psI, 0.0)
            nc.scalar.sqrt(sC, sC)
            nc.scalar.sqrt(sI, sI)
            # sub on GpSimd: on the in-order DVE it would wait for the
            # ScalarE sqrts and head-of-line-block the next iteration's
            # max, throttling wg PSUM recycling to sqrt pace
            nc.gpsimd.tensor_sub(w_sb[:, kt, osl], sC, sI)

        def main_group(oh, bt, xt):
            """16-MM PSUM accumulation for one [128,512] output tile."""
            osl = slice(oh * 512, (oh + 1) * 512)
            ot = opool.tile([128, 512], fp32, name="ot")
            # pre-touch: absorbs the out-DMA slot-release wait on ScalarE
            # so the drains stay within the HW sync-wait slot budget
            # (GpSimd tried and reverted: its per-instruction overhead
            # delays the dependent DVE drain and stalls PSUM recycling)
            nc.scalar.mul(ot[0:1, 0:1], ot[0:1, 0:1], 0.0)
            ps = mps.tile([128, 512], fp32, tag="ps", bufs=4)
            for kt in range(K_TILES):
                nc.tensor.matmul(
                    ps,
                    xt[:, kt * 128 : (kt + 1) * 128],
                    w_sb[:, kt, osl],
                    start=(kt == 0),
                    stop=(kt == K_TILES - 1),
                )
            # drain with bias add on DVE (was: PE matmul + scalar copy)
            nc.vector.tensor_add(ot, ps, bias_sb[:, osl])
            nc.sync.dma_start(out=out[bt * 128 : (bt + 1) * 128, osl], in_=ot)

        def x_tiles(half):
            xts = []
            for i in range(HB):
                xt = xpool.tile([128, NUM_IN], mmdt, name="xt")
                nc.sync.dma_start(out=xt, in_=xT[half * HB + i])
                xts.append(xt)
            return xts

        # Issue order is the PE execution order (in-order engine queues):
        #  1. wg oh=0, PE duty padded with filler MMs
        #  2. wg oh=1 interleaved with half0/oh=0 main groups - the main MMs
        #     fill the PE while the oh=1 elementwise chain drains, and the
        #     oh=0 groups only need oh=0 weights (no head-of-line blocking)
        #  3. remaining main groups (all weights ready by then)
        xts0 = x_tiles(0)
        for kt in range(K_TILES):
            wg_iter(0, kt, n_dummy=1)
        for j in range(K_TILES):
            wg_iter(1, j, n_dummy=2 if j < 3 else 0)
            main_group(0, j, xts0[j])
        for i in range(HB):
            main_group(1, i, xts0[i])
        xts1 = x_tiles(1)
        for oh in range(O_HALves):
            for i in range(HB):
                if oh == O_HALves - 1 and i == HB - 1:
                    continue  # final group handled below, column-split
                main_group(oh, HB + i, xts1[i])

        # Final group split into two 256-wide accumulations: the first
        # half's drain + out-DMA overlap the second half's matmuls, and the
        # terminal DMA (whose completion semaphore gates NEFF teardown) is
        # half-size - shaves ~1us off the post-last-matmul tail.
        bt = B_TILES - 1
        xt = xts1[HB - 1]
        ot = opool.tile([128, 512], fp32, name="ot")
        nc.scalar.mul(ot[0:1, 0:1], ot[0:1, 0:1], 0.0)
        for hc in range(2):
            csl = slice(512 + hc * 256, 512 + (hc + 1) * 256)
            ps = mps.tile([128, 512], fp32, tag="ps", bufs=4)
            for kt in range(K_TILES):
                nc.tensor.matmul(
                    ps[:, 0:256],
                    xt[:, kt * 128 : (kt + 1) * 128],
                    w_sb[:, kt, csl],
                    start=(kt == 0),
                    stop=(kt == K_TILES - 1),
                )
            osl_ot = slice(hc * 256, (hc + 1) * 256)
            nc.vector.tensor_add(ot[:, osl_ot], ps[:, 0:256], bias_sb[:, csl])
            nc.sync.dma_start(
                out=out[bt * 128 : (bt + 1) * 128, csl], in_=ot[:, osl_ot]
            )

    nc.finalize()
    return nc


def _hi_lo3(v):
    """Split fp32 array into 3 bf16 terms with v ~ h+l+l2 to ~2^-27."""
    import ml_dtypes

    bf = ml_dtypes.bfloat16
    h = v.astype(bf)
    r = v - h.astype(np.float32)
    l = r.astype(bf)
    l2 = (r - l.astype(np.float32)).astype(bf)
    return h, l, l2


def _aug_pair(p, q):
    """Augmented bf16 row blocks for a (in, [N,5]) and b (out, [M,5]) such
    that A.T @ B ~= ||a||^2 - 2 a.b + ||b||^2 in one bf16 matmul pass with
    ~2^-27 split error (the residual error is fp32 PSUM accumulation).

    a ~ ah+al+al2 (bf16 each); kept cross terms: ah.bh, ah.bl, al.bh,
    ah.bl2, al.bl, al2.bh; norms split 3-way against ones rows.
    """
    import ml_dtypes

    bf = ml_dtypes.bfloat16
    n, m = p.shape[0], q.shape[0]
    na = (p.astype(np.float64) ** 2).sum(1).astype(np.float32)
    nb = (q.astype(np.float64) ** 2).sum(1).astype(np.float32)
    ah, al, al2 = _hi_lo3(p)
    bh, bl, bl2 = _hi_lo3(q)
    nah, nal, nal2 = _hi_lo3(na)
    nbh, nbl, nbl2 = _hi_lo3(nb)
    ones_n = np.ones((1, n), bf)
    ones_m = np.ones((1, m), bf)
    A = np.concatenate(
        [ah.T, ah.T, al.T, ah.T, al.T, al2.T,
         nah[None, :], nal[None, :], nal2[None, :], ones_n, ones_n, ones_n], 0
    ).astype(bf)
    B = np.concatenate(
        [-2 * bh.T, -2 * bl.T, -2 * bh.T, -2 * bl2.T, -2 * bl.T, -2 * bh.T,
         ones_m, ones_m, ones_m, nbh[None, :], nbl[None, :], nbl2[None, :]], 0
    ).astype(bf)
    assert A.shape == (KAUG, n) and B.shape == (KAUG, m)
    return A, B


def _prep_inputs(inputs, init_in_pos, init_out_pos, in_pos, out_pos, biases,
                 mm_np_dt=np.float16):
    x = np.ascontiguousarray(np.asarray(inputs, dtype=np.float32))
    a = np.asarray(in_pos, dtype=np.float32).reshape(NUM_IN, SD)
    a0 = np.asarray(init_in_pos, dtype=np.float32).reshape(NUM_IN, SD)
    b = np.asarray(out_pos, dtype=np.float32).reshape(NUM_OUT, SD)
    b0 = np.asarray(init_out_pos, dtype=np.float32).reshape(NUM_OUT, SD)
    bias = np.asarray(biases, dtype=np.float32).reshape(NUM_OUT)

    # [bt, p, kt*128+b'] = x[bt*128+b', kt*128+p]
    xT = np.ascontiguousarray(
        x.reshape(B_TILES, 128, K_TILES, 128).transpose(0, 3, 2, 1).astype(mm_np_dt)
    ).reshape(B_TILES, 128, NUM_IN)

    A_C, B_C_full = _aug_pair(a, b)
    A_I, B_I_full = _aug_pair(a0, b0)

    in_maps = []
    for c in range(N_CORES):
        sl = slice(c * O_SHARD, (c + 1) * O_SHARD)
        ab = np.ascontiguousarray(
            np.concatenate([A_C, A_I, B_C_full[:, sl], B_I_full[:, sl]], axis=1)
        )
        bias_bc = np.ascontiguousarray(
            np.broadcast_to(bias[sl][None, :], (128, O_SHARD)).astype(np.float32)
        )
        in_maps.append({"xT": xT, "ab": ab, "bias": bias_bc})
    return in_maps


def _run(in_maps, trace=False):
    from concourse.bass_utils import run_bass_kernel_spmd

    if "nc" not in _CACHE:
        _CACHE["nc"] = _build_bass()
    nc = _CACHE["nc"]
    res = run_bass_kernel_spmd(
        nc, in_maps, core_ids=list(range(N_CORES)), trace=trace
    )
    outs = [r["out"] for r in res.results]
    return np.concatenate(outs, axis=1), res


def kernel(**inputs) -> np.ndarray:
    in_maps = _prep_inputs(**inputs)
    out, _ = _run(in_maps, trace=bool(os.environ.get("MESHFC_TRACE")))
    return out
